# revision 29
# baseline (speedup 1.0000x reference)
"""SEIR Euler trajectory kernel for 8 TRN2 NeuronCores.

Fast path (used for the actual problem instance, where sigma == 0.0):
with sigma = 0 the Euler recurrence decouples in closed form:

    a3   = 1 - gamma/2,  c1 = beta/2,  c3 = gamma/2,  u = c1*I0
    I(t) = I0 * a3^t
    R(t) = R0 + c3*I0 * g(t),          g(t) = (1-a3^t)/(1-a3)
    S(t) = S0 * exp(-sum_k c_k(t) u^k), c_k(t) = (1-a3^{kt})/(k(1-a3^k))
    E(t) = E0 + S0 - S(t)              (exact telescoping of the S->E flux)

so the whole (T x B) trajectory is rank-structured: w(t,b) = -sum_k c_k(t)
u^k(b) is a rank-K matmul (PE engine), I and R are rank-1/rank-2 matmuls,
and only exp(w) (ACT) plus two wide elementwise ops (DVE mult, Pool
subtract) remain.  Each instruction covers 128 timesteps x 512 batch
elements, so the kernel is DMA-bound (~64MB of trajectory written per
core) instead of instruction-overhead bound.

Layout per core (BS=4096 batch elements): chunk = 128 timesteps (PSUM/SBUF
partition dim) x 512 batch columns; 8 batch-chunks x T/128 time-chunks.
DRAM out is 4 compartment planes [4, T, BS]; the host interleaves to
(T*B, 4).

The general path (sigma != 0) falls back to the per-step Tile kernel from
the previous revision (kept verbatim at the bottom of this file).

Toolchain constraint: this container's walrus build rejects instructions
carrying >2 semaphore waits.  The fast path uses hand-placed semaphores
with at most 2 waits per instruction (standalone wait_ge instructions).
"""

import sys

sys.path.insert(0, "/opt/trn_rl_repo")

from contextlib import ExitStack

import numpy as np

import concourse.bass as bass
import concourse.tile as tile
import concourse.tile_sem_assignment as _tsa
from concourse import mybir
from concourse.bass_utils import run_bass_kernel_spmd

# Fallback tile path needs single DMA sem lanes (see fallback notes below).
_tsa.NUM_HWDGE_SEMS = 1
_tsa.NUM_SWDGE_GLOBAL_SEMS = 1

T = 1024
B = 32768
NCORES = 8
BS = B // NCORES  # 4096
P = 128
F = 512  # batch columns per chunk
NBCH = BS // F  # 8 batch chunks
K = 5  # series terms for S
C = 4
G = BS // P  # fallback layout constant

TRACE = False

f32 = mybir.dt.float32
bf16 = mybir.dt.bfloat16
mult = mybir.AluOpType.mult
add = mybir.AluOpType.add
subtract = mybir.AluOpType.subtract
EXP = mybir.ActivationFunctionType.Exp


def _build_fast(t_total=T, passes=1, nbuf=8, e_dma="sp", irslots=2):
    """Closed-form (sigma==0) kernel: one NeuronCore's program."""
    NT = t_total // P
    nchunks = NBCH * NT
    nc = bass.Bass(trn_type="TRN2")

    ctab = nc.dram_tensor("ctab", [K, t_total], f32, kind="ExternalInput")
    a3t = nc.dram_tensor("a3t", [1, t_total], f32, kind="ExternalInput")
    rstat = nc.dram_tensor("rstat", [2, t_total], f32, kind="ExternalInput")
    urows = nc.dram_tensor("urows", [K, BS], f32, kind="ExternalInput")
    iro = nc.dram_tensor("iro", [2, BS], f32, kind="ExternalInput")
    brows = nc.dram_tensor("brows", [2, BS], f32, kind="ExternalInput")
    out = nc.dram_tensor("out", [C, t_total, BS], f32, kind="ExternalOutput")

    es = ExitStack()
    with es:
        # --- persistent SBUF ---
        sb = lambda name, shape: es.enter_context(nc.sbuf_tensor(name, shape, f32))
        sbh = lambda name, shape: es.enter_context(
            nc.sbuf_tensor(name, shape, bf16)
        )
        ps = lambda name, shape: es.enter_context(nc.psum_tensor(name, shape, f32))
        ctab_sb = sb("ctab_sb", [K, t_total])
        a3t_sb = sb("a3t_sb", [1, t_total])
        rstat_sb = sb("rstat_sb", [2, t_total])
        NBUF = nbuf  # staging pipeline depth (hides cross-engine+DMA latency)
        urows_sb = [sb(f"urows_sb{i}", [K, F]) for i in range(2)]
        iro_sb = [sb(f"iro_sb{i}", [2, F]) for i in range(2)]
        s0r = [sb(f"s0r{i}", [P, F]) for i in range(2)]
        wr = [sb(f"wr{i}", [P, F]) for i in range(2)]
        exps = [sb(f"exps{i}", [P, F]) for i in range(NBUF)]
        # staging: cols 0:F = S, F:2F = E  /  0:F = I, F:2F = R
        sesb = [sb(f"sesb{i}", [P, 2 * F]) for i in range(NBUF)]
        irsb = [sb(f"irsb{i}", [P, 2 * F]) for i in range(NBUF)]
        # PSUM: w in its own bank; I and R in one 2-bank tile
        wps = [ps(f"wps{i}", [P, F]) for i in range(2)]
        irps = [ps(f"irps{i}", [P, 2 * F]) for i in range(irslots)]

        PESEM = es.enter_context(nc.semaphore("pesem"))
        ACTSEM = es.enter_context(nc.semaphore("actsem"))
        DVESEM = es.enter_context(nc.semaphore("dvesem"))
        POOLSEM = es.enter_context(nc.semaphore("poolsem"))
        # DMA completions are unordered; per-buffer-slot sems keep every
        # wait unambiguous (same-slot DMAs are serialized by data deps).
        DMAIN = [es.enter_context(nc.semaphore(f"dmain{i}")) for i in range(2)]
        DMASE = [es.enter_context(nc.semaphore(f"dmase{i}")) for i in range(NBUF)]
        DMAIR = [es.enter_context(nc.semaphore(f"dmair{i}")) for i in range(NBUF)]
        DMAEP = [es.enter_context(nc.semaphore(f"dmaep{i}")) for i in range(NBUF)]

        chunks = [
            (jb, jt)
            for _ in range(passes)
            for jb in range(NBCH)
            for jt in range(NT)
        ]

        blk = es.enter_context(nc.Block(no_gpsimd_drain=True))

        @blk.tensor
        def _(pe):
            for c, (jb, jt) in enumerate(chunks):
                ts = slice(jt * P, (jt + 1) * P)
                if jt == 0 and c < nchunks:
                    # inputs for batch-chunk jb (buffer set jb%2) loaded
                    pe.wait_ge(
                        DMAIN[jb % 2],
                        64 * (jb // 2 + 1) + (48 if jb % 2 == 0 else 0),
                    )
                if c >= 2:
                    # PSUM WAR: exp(c-2) + bigcopy(c-irslots) done
                    pe.wait_ge(
                        ACTSEM,
                        max(3 * (c - 2) + 1, 3 * (c - irslots) + 3),
                    )
                nc.tensor.matmul(
                    wps[c % 2][:, :],
                    ctab_sb[0:K, ts],
                    urows_sb[jb % 2][0:K, :],
                    start=True,
                    stop=True,
                ).then_inc(PESEM, 1)
                nc.tensor.matmul(
                    irps[c % irslots][:, 0:F],
                    a3t_sb[0:1, ts],
                    iro_sb[jb % 2][0:1, :],
                    start=True,
                    stop=True,
                ).then_inc(PESEM, 1)
                nc.tensor.matmul(
                    irps[c % irslots][:, F : 2 * F],
                    rstat_sb[0:2, ts],
                    iro_sb[jb % 2][0:2, :],
                    start=True,
                    stop=True,
                ).then_inc(PESEM, 1)

        @blk.scalar
        def _(act):
            for c, (jb, jt) in enumerate(chunks):
                trange = slice(jt * P, (jt + 1) * P)
                brange = slice(jb * F, (jb + 1) * F)
                act.wait_ge(PESEM, 3 * c + 1)
                if c >= NBUF:
                    # exps WAR: DVE smul of chunk c-NBUF done
                    act.wait_ge(DVESEM, 2 * (c - NBUF) + 1)
                nc.scalar.activation(
                    exps[c % NBUF][:, :], wps[c % 2][:, :], EXP
                ).then_inc(ACTSEM, 1)
                act.wait_ge(PESEM, 3 * (c + 1))
                if c >= NBUF:
                    # irsb WAR: IR out-DMA of chunk c-NBUF done
                    act.wait_ge(DMAIR[c % NBUF], 16 * (c // NBUF))
                nc.scalar.copy(
                    irsb[c % NBUF][:, 0 : 2 * F], irps[c % irslots][:, 0 : 2 * F]
                ).then_inc(ACTSEM, 2)
                # I/R out-DMA on the ACT HWDGE queue.
                act.wait_ge(ACTSEM, 3 * (c + 1))
                act.dma_start(
                    out=out[2:4, trange, brange].rearrange("c t b -> t c b"),
                    in_=irsb[c % NBUF][:, :].rearrange("t (c b) -> t c b", c=2),
                ).then_inc(DMAIR[c % NBUF], 16)

        @blk.vector
        def _(dve):
            for c, (jb, jt) in enumerate(chunks):
                dve.wait_ge(ACTSEM, 3 * c + 1)
                if c >= NBUF:
                    # sesb WAR: SE out-DMA of chunk c-NBUF done
                    dve.wait_ge(DMASE[c % NBUF], 16 * (c // NBUF))
                nc.vector.tensor_mul(
                    sesb[c % NBUF][:, 0:F], exps[c % NBUF][:, :], s0r[jb % 2][:, :]
                ).then_inc(DVESEM, 1)
                # E = W - S (same-engine wait: deep DVE pipeline RAW)
                dve.wait_ge(DVESEM, 2 * c + 1)
                if e_dma == "pool" and c >= NBUF:
                    # sesb E-half WAR: pool E-DMA of chunk c-NBUF done
                    dve.wait_ge(DMAEP[c % NBUF], 16 * (c // NBUF))
                nc.vector.tensor_sub(
                    sesb[c % NBUF][:, F : 2 * F],
                    wr[jb % 2][:, :],
                    sesb[c % NBUF][:, 0:F],
                ).then_inc(DVESEM, 1)

        if e_dma == "pool":

            @blk.gpsimd
            def _(pool):
                for c, (jb, jt) in enumerate(chunks):
                    trange = slice(jt * P, (jt + 1) * P)
                    brange = slice(jb * F, (jb + 1) * F)
                    pool.wait_ge(DVESEM, 2 * (c + 1))
                    pool.dma_start(
                        out=out[1, trange, brange],
                        in_=sesb[c % NBUF][:, F : 2 * F],
                    ).then_inc(DMAEP[c % NBUF], 16)

        @blk.sync
        def _(sp):
            def bcast(dst, src_row, sem):
                src = src_row
                ap = bass.AP(
                    tensor=src.tensor,
                    offset=src.offset,
                    ap=[[0, P]] + src.ap[1:],
                )
                sp.dma_start(out=dst, in_=ap).then_inc(sem, 16)

            def load_set(jb):
                bf = jb % 2
                sem = DMAIN[bf]
                bs = slice(jb * F, (jb + 1) * F)
                sp.dma_start(out=urows_sb[bf][:, :], in_=urows[:, bs]).then_inc(
                    sem, 16
                )
                sp.dma_start(out=iro_sb[bf][:, :], in_=iro[:, bs]).then_inc(sem, 16)
                bcast(s0r[bf][:, :], brows[0:1, bs], sem)
                bcast(wr[bf][:, :], brows[1:2, bs], sem)

            sp.dma_start(out=ctab_sb[:, :], in_=ctab[:, :]).then_inc(DMAIN[0], 16)
            sp.dma_start(out=a3t_sb[:, :], in_=a3t[:, :]).then_inc(DMAIN[0], 16)
            sp.dma_start(out=rstat_sb[:, :], in_=rstat[:, :]).then_inc(DMAIN[0], 16)
            load_set(0)

            for c, (jb, jt) in enumerate(chunks):
                if jt == 0 and jb + 1 < NBCH and c < nchunks:
                    if jb >= 1:
                        # buffer set (jb+1)%2 == (jb-1)%2 free: consumers done
                        sp.wait_ge(DVESEM, 2 * NT * jb)
                        sp.wait_ge(PESEM, 3 * NT * jb)
                    load_set(jb + 1)
                trange = slice(jt * P, (jt + 1) * P)
                brange = slice(jb * F, (jb + 1) * F)
                if e_dma == "pool":
                    sp.wait_ge(DVESEM, 2 * c + 1)
                    sp.dma_start(
                        out=out[0, trange, brange],
                        in_=sesb[c % NBUF][:, 0:F],
                    ).then_inc(DMASE[c % NBUF], 16)
                else:
                    sp.wait_ge(DVESEM, 2 * (c + 1))
                    sp.dma_start(
                        out=out[0:2, trange, brange].rearrange("c t b -> t c b"),
                        in_=sesb[c % NBUF][:, :].rearrange("t (c b) -> t c b", c=2),
                    ).then_inc(DMASE[c % NBUF], 16)

            ntot = nchunks * passes
            for i in range(NBUF):
                n_i = ntot // NBUF + (1 if ntot % NBUF > i else 0)
                sp.wait_ge(DMASE[i], 16 * n_i)
                sp.wait_ge(DMAIR[i], 16 * n_i)
                if e_dma == "pool":
                    sp.wait_ge(DMAEP[i], 16 * n_i)

    return nc


def _fast_inputs(initial_shard, beta, gamma, t_total):
    """Host-side prep of the closed-form coefficient arrays (float64)."""
    S0, E0, I0, R0 = (initial_shard[i].astype(np.float64) for i in range(4))
    c1 = 0.5 * np.float64(beta)
    c3 = 0.5 * np.float64(gamma)
    a3 = 1.0 - c3
    t = np.arange(t_total, dtype=np.float64)
    a3t = a3**t
    gt = (1.0 - a3t) / (1.0 - a3)
    ck = np.stack(
        [(1.0 - a3 ** (k * t)) / (k * (1.0 - a3**k)) for k in range(1, K + 1)]
    )
    u = c1 * I0
    return {
        "ctab": (-ck).astype(np.float32),
        "a3t": a3t[None].astype(np.float32),
        "rstat": np.stack([c3 * gt, np.ones_like(gt)]).astype(np.float32),
        "urows": np.stack([u**k for k in range(1, K + 1)]).astype(np.float32),
        "iro": np.stack([I0, R0]).astype(np.float32),
        "brows": np.stack([S0, S0 + E0]).astype(np.float32),
    }


def device_out_to_full(outs, t_total=T):
    """[ncores][4, t, BS] planes -> (t, ncores*BS, 4)."""
    n = len(outs)
    full = np.empty((t_total, n * BS, C), dtype=np.float32)
    for i in range(n):
        full[:, i * BS : (i + 1) * BS, :] = np.asarray(outs[i]).transpose(1, 2, 0)
    return full


_nc_fast = None
_nc_fallback = None


def kernel(initial, beta, gamma, sigma, t):
    global _nc_fast, _nc_fallback
    assert int(t) == T
    initial = np.ascontiguousarray(np.asarray(initial, dtype=np.float32))
    beta = np.asarray(beta, dtype=np.float32).reshape(1)
    gamma = np.asarray(gamma, dtype=np.float32).reshape(1)
    sigma = np.asarray(sigma, dtype=np.float32).reshape(1)
    assert initial.shape == (C, B)

    use_fast = (
        float(sigma[0]) == 0.0
        and float(gamma[0]) > 1e-6
        and float(beta[0]) * 0.5 * float(np.abs(initial[2]).max()) < 0.5
    )

    if use_fast:
        if _nc_fast is None:
            _nc_fast = _build_fast()
        in_maps = []
        for i in range(NCORES):
            shard = initial[:, i * BS : (i + 1) * BS]
            in_maps.append(_fast_inputs(shard, beta[0], gamma[0], T))
        res = None
        if TRACE:
            try:
                res = run_bass_kernel_spmd(
                    _nc_fast, in_maps, core_ids=list(range(NCORES)), trace=True
                )
            except Exception:
                res = None  # no NTFF hook in this container
        if res is None:
            res = run_bass_kernel_spmd(
                _nc_fast, in_maps, core_ids=list(range(NCORES)), trace=False
            )
        if TRACE and res.exec_time_ns is not None:
            print(f"HW exec time: {res.exec_time_ns} ns")
        full = device_out_to_full([res.results[i]["out"] for i in range(NCORES)])
        return full.reshape(T * B, C)

    # ---------------- general fallback (sigma != 0) ----------------
    if _nc_fallback is None:
        _nc_fallback = _build_fallback()
    in_maps = []
    for i in range(NCORES):
        shard = np.ascontiguousarray(initial[:, i * BS : (i + 1) * BS])
        in_maps.append(
            {"initial": shard, "beta": beta, "gamma": gamma, "sigma": sigma}
        )
    res = run_bass_kernel_spmd(
        _nc_fallback, in_maps, core_ids=list(range(NCORES)), trace=TRACE
    )
    if TRACE and res.exec_time_ns is not None:
        print(f"HW exec time: {res.exec_time_ns} ns")
    full = np.empty((T, NCORES, BS, C), dtype=np.float32)
    for i in range(NCORES):
        full[:, i] = res.results[i]["out"].reshape(T, P, G, C).reshape(T, BS, C)
    return full.reshape(T * B, C)


# ======================================================================
# General-path (sigma != 0) per-step Euler kernel: Tile framework, DVE
# only, one DMA lane.  Unchanged from the previous revision; see its
# docstring there for the walrus sync-wait legalization notes.
# ======================================================================

FREE = G * C
KBLK = 16
NBLK = T // KBLK


def _build_fallback(t_total=T, w_engine="dve", passes=1, chain=False):
    nblk = t_total // KBLK
    nc = bass.Bass(trn_type="TRN2")
    init = nc.dram_tensor("initial", [C, BS], f32, kind="ExternalInput")
    beta = nc.dram_tensor("beta", [1], f32, kind="ExternalInput")
    gamma = nc.dram_tensor("gamma", [1], f32, kind="ExternalInput")
    sigma = nc.dram_tensor("sigma", [1], f32, kind="ExternalInput")
    out = nc.dram_tensor("out", [t_total, P, FREE], f32, kind="ExternalOutput")
    chain_in = chain_out = None
    if chain:
        chain_in = nc.dram_tensor("chain", [1, 1], f32, kind="ExternalInput")
        chain_out = nc.dram_tensor("chain_out", [1, 1], f32, kind="ExternalOutput")

    with tile.TileContext(nc) as tc:
        with (
            tc.tile_pool(name="consts", bufs=1) as consts,
            tc.tile_pool(name="stage", bufs=3) as stagep,
            tc.tile_pool(name="scratch", bufs=4) as scratch,
        ):
            bt = consts.tile([P, 1], f32, tag="bt")
            gt = consts.tile([P, 1], f32, tag="gt")
            st = consts.tile([P, 1], f32, tag="st")
            for dst, src in ((bt, beta), (gt, gamma), (st, sigma)):
                src_ap = src[:]
                bcast = bass.AP(
                    tensor=src_ap.tensor,
                    offset=src_ap.offset,
                    ap=[[0, P], [1, 1]],
                )
                nc.sync.dma_start(out=dst[:, :], in_=bcast)

            c1t = consts.tile([P, 1], f32, tag="c1")
            c2t = consts.tile([P, 1], f32, tag="c2")
            c3t = consts.tile([P, 1], f32, tag="c3")
            a2t = consts.tile([P, 1], f32, tag="a2")
            a3t = consts.tile([P, 1], f32, tag="a3")
            nc.vector.tensor_scalar_mul(c1t[:, :], bt[:, :], 0.5)
            nc.vector.tensor_scalar_mul(c2t[:, :], st[:, :], 0.5)
            nc.vector.tensor_scalar_mul(c3t[:, :], gt[:, :], 0.5)
            nc.vector.tensor_scalar(a2t[:, :], st[:, :], -0.5, 1.0, mult, add)
            nc.vector.tensor_scalar(a3t[:, :], gt[:, :], -0.5, 1.0, mult, add)
            c1 = c1t[:, 0:1]
            c2 = c2t[:, 0:1]
            c3 = c3t[:, 0:1]
            a2 = a2t[:, 0:1]
            a3 = a3t[:, 0:1]

            cur = stagep.tile([P, KBLK * FREE], f32, tag="stage")
            r = cur[:, :].rearrange("p (k g c) -> p k c g", k=KBLK, g=G, c=C)
            tmp0 = consts.tile([P, FREE], f32, tag="init_tmp")
            nc.sync.dma_start(
                out=tmp0[:, :].rearrange("p (c g) -> p c g", c=C),
                in_=init[:, :].rearrange("c (p g) -> p c g", p=P),
            )
            nc.vector.tensor_copy(
                out=r[:, 0, :, :],
                in_=tmp0[:, :].rearrange("p (c g) -> p c g", c=C),
            )

            prev_r, prev_k = r, 0
            first = True
            for blk in range(nblk * passes):
                blk_out = blk % nblk
                if not first:
                    cur = stagep.tile([P, KBLK * FREE], f32, tag="stage")
                    r = cur[:, :].rearrange(
                        "p (k g c) -> p k c g", k=KBLK, g=G, c=C
                    )
                ks = range(1, KBLK) if first else range(KBLK)
                first = False
                for k in ks:
                    Sp = prev_r[:, prev_k, 0, :]
                    Ep = prev_r[:, prev_k, 1, :]
                    Ip = prev_r[:, prev_k, 2, :]
                    Rp = prev_r[:, prev_k, 3, :]
                    w = scratch.tile([P, G], f32, tag="w")
                    d1 = scratch.tile([P, G], f32, tag="d1")
                    if w_engine == "act":
                        nc.scalar.mul(w[:, :], Ep, c2)
                    elif w_engine == "pool":
                        nc.gpsimd.tensor_scalar_mul(w[:, :], Ep, c2)
                    else:
                        nc.vector.tensor_scalar_mul(w[:, :], Ep, c2)
                    nc.vector.scalar_tensor_tensor(d1[:, :], Sp, c1, Ip, mult, mult)
                    nc.vector.tensor_sub(r[:, k, 0, :], Sp, d1[:, :])
                    nc.vector.scalar_tensor_tensor(
                        r[:, k, 1, :], Ep, a2, d1[:, :], mult, add
                    )
                    nc.vector.scalar_tensor_tensor(
                        r[:, k, 3, :], Ip, c3, Rp, mult, add
                    )
                    nc.vector.scalar_tensor_tensor(
                        r[:, k, 2, :], Ip, a3, w[:, :], mult, add
                    )
                    prev_r, prev_k = r, k
                dview = out[blk_out * KBLK : (blk_out + 1) * KBLK, :, :].rearrange(
                    "k p f -> p k f"
                )
                sview = cur[:, :].rearrange("p (k f) -> p k f", k=KBLK)
                nc.sync.dma_start(out=dview, in_=sview)

            if chain:
                cht = consts.tile([1, 1], f32, tag="chain")
                nc.sync.dma_start(out=cht[:, :], in_=chain_in[:, :])
                chv = consts.tile([1, 1], f32, tag="chainv")
                last_elem = r[0:1, KBLK - 1, 2, 0:1]
                nc.vector.tensor_scalar_mul(chv[:, :], last_elem, cht[0:1, 0:1])
                nc.sync.dma_start(out=chain_out[:, :], in_=chv[:, :])

    # walrus 2-sync-wait legalization (see previous revision for rationale)
    for bb in nc.m.functions[0].blocks:
        for ins in bb.instructions:
            si = ins.sync_info
            if si is None:
                continue
            ow = si.on_wait
            if not ow or len(ow) < 2:
                continue
            kind = ins.__class__.__name__
            eng = str(ins.engine).rsplit(".", 1)[-1]
            if kind == "InstDMACopy":
                new_w = [
                    w
                    for w in ow
                    if not (
                        w.ant_name.startswith("DMAHW")
                        or w.ant_name.startswith("DMASW")
                    )
                ]
            elif kind == "InstDrain":
                dma_w = [w for w in ow if w.ant_name.startswith("DMA")]
                new_w = dma_w[-1:] if dma_w else ow[-1:]
            else:
                new_w = [
                    w
                    for w in ow
                    if not (
                        w.wait_mode == "sem-ge-imm"
                        and w.ant_name.split("_")[0] == eng
                    )
                ]
            if len(new_w) < len(ow):
                si.on_wait = new_w
                ins.sync_info = si
    return nc


if __name__ == "__main__":
    rng = np.random.default_rng(0)
    ini = rng.random((C, B), dtype=np.float32)
    be, ga = (rng.random(1, dtype=np.float32) for _ in range(2))
    si = np.zeros(1, dtype=np.float32)
    outv = kernel(ini, be, ga, si, T)
    print("ran, out shape", outv.shape, outv[:4])
